# revision 28
# baseline (speedup 1.0000x reference)
"""Trainium2 Bass kernel for the SE(3) deformation model.

reference math (per point):
    w, v, pivot, t = split(network_output, 4)
    theta = |w| + eps ; wn = w/theta ; vn = v/theta
    R = I + sin(theta) K + (1-cos(theta)) K^2          (K = skew(wn))
    p = (theta I + (1-cos) K + (theta-sin) K^2) vn
    out = R (x + pivot) + p - pivot + t - x

Exact rewrite used here (K~ = skew(w) unnormalized, n2 = |w|^2):
    u  = x + pivot
    k1 = sin(theta)/theta ; k2 = (1-cos(theta))/theta^2
    sg = (theta - sin(theta))/theta^3
    out = K~ (k1 u + k2 v) + K~^2 (k2 u + sg v) + v + t
    with K~^2 a = w (w . a) - n2 a
(algebraically identical to the reference; skew(a)^2 = a a^T - |a|^2 I).

Implementation notes (driven by measured HW rates):
  - Data is processed in chunks of 128*F points; fp16 "planar" layout
    [w0|w1|w2] (three [128,F] planes in one [128,3F] tile) makes every
    bulk op a contiguous step-1 fp16 op -> DVE 2x mode.
  - Cross product via extended tiles [p0|p1|p2|p0|p1]: rotated component
    views are then contiguous [3F] slices, so w x g is 3 full-width ops.
  - Per-point coefficients [128,F] are broadcast over planes with
    step-0 middle-dim APs (measured: free on DVE).
  - Inputs are cast f32->f16 during the DMA load (SWDGE), output cast
    f16->f32 during the store. HBM traffic is unchanged (f32 on the
    DRAM side); SBUF tiles halve.
  - The scalar chain stays f32: theta-sin(theta) cancels catastrophically
    in f16, and 1/theta^2, 1/theta^3 overflow f16 range for the smallest
    theta in a 4M-point gaussian sample.
  - sin inputs are range-reduced with the ADD_RANGE_WRAP custom DVE op
    (ACT sin domain is [-pi,pi]); cos(x) = -sin(x - pi/2) keeps the
    wrapped argument in range for theta < 3.5*pi.
  - Engine split keeps ACT on 1-input work (activations, planar
    deinterleave copy), GPSIMD on three big ops, DVE on the rest.
"""

import math

import numpy as np

import concourse.bacc as bacc
import concourse.mybir as mybir
import concourse.tile as tile
from concourse.alu_op_type import AluOpType
from concourse.bass_utils import run_bass_kernel_spmd

AFT = mybir.ActivationFunctionType
F32 = mybir.dt.float32
F16 = mybir.dt.float16

N_TOTAL = 4194304
NCORES = 8
NPC = N_TOTAL // NCORES  # 524288 points per core
P = 128
F_DEF = 512  # points per partition per chunk
EPS = 1e-6
HALF_PI = float(np.pi / 2)


def build_nc(npc: int = NPC, f: int = F_DEF):
    nchunks = npc // (P * f)
    assert nchunks * P * f == npc

    nc = bacc.Bacc("TRN2", target_bir_lowering=False, debug=False)

    pos = nc.dram_tensor("pos", [npc, 3], F32, kind="ExternalInput")
    net = nc.dram_tensor("net", [npc, 12], F32, kind="ExternalInput")
    out = nc.dram_tensor("out", [npc, 3], F32, kind="ExternalOutput")

    pos_r = pos.ap().rearrange("(n p f) c -> n p (f c)", p=P, f=f)
    net_r = net.ap().rearrange("(n p f) c -> n p (f c)", p=P, f=f)
    out_r = out.ap().rearrange("(n p f) c -> n p (f c)", p=P, f=f)

    V = nc.vector
    G = nc.gpsimd
    S = nc.scalar
    mul, add, sub = AluOpType.mult, AluOpType.add, AluOpType.subtract

    with tile.TileContext(nc) as tc:
        with (
            tc.tile_pool(name="io", bufs=2) as io,
            tc.tile_pool(name="vec", bufs=2) as vec,
            tc.tile_pool(name="sc", bufs=2) as sc,
            tc.tile_pool(name="ps", bufs=2, space="PSUM") as ps,
        ):
            for i in range(nchunks):
                x32 = io.tile([P, 3 * f], F32, tag="x", name="x32")
                net32 = io.tile([P, 12 * f], F32, tag="net", name="net32")
                o16 = io.tile([P, 3 * f], F16, tag="o", name="o16")

                # HWDGE loads (keeps the GPSIMD queue free of DGE work);
                # the deinterleave ops below do the f32->f16 cast for free.
                nc.sync.dma_start(out=x32[:], in_=pos_r[i])
                nc.sync.dma_start(out=net32[:], in_=net_r[i])

                # planar-order views of the interleaved inputs:
                # [P, comp, point] with per-point stride 12 (net) / 3 (pos)
                netp = net32[:].rearrange("p (f c) -> p c f", c=12)
                xp = x32[:].rearrange("p (f c) -> p c f", c=3)

                def v3(t):
                    return t[:, 0 : 3 * f].rearrange("p (c f) -> p c f", c=3)

                # planar tiles ([3F] = concatenated planes); w,g extended to 5F
                wx_t = vec.tile([P, 5 * f], F16, tag="wx", name="wx_t")
                gx_t = vec.tile([P, 5 * f], F16, tag="gx", name="gx_t")
                v16 = vec.tile([P, 3 * f], F16, tag="v16", name="v16")
                u16 = vec.tile([P, 3 * f], F16, tag="u16", name="u16")
                h16 = vec.tile([P, 3 * f], F16, tag="h16", name="h16")
                sq16 = vec.tile([P, 3 * f], F16, tag="sq16", name="sq16")
                pr16 = vec.tile([P, 3 * f], F16, tag="pr16", name="pr16")
                cr16 = vec.tile([P, 3 * f], F16, tag="cr16", name="cr16")
                a1_t = vec.tile([P, 3 * f], F16, tag="a1", name="a1_t")
                m1_t = vec.tile([P, 3 * f], F16, tag="m1", name="m1_t")
                m2_t = vec.tile([P, 3 * f], F16, tag="sq16", name="m2_t")

                def stile(tag, dt=F32, pool=None):
                    return (pool or sc).tile([P, f], dt, tag=tag, name=tag + "_t")

                n2_16 = stile("n2h", F16)
                r32 = stile("r32", pool=ps)
                th = stile("th", pool=ps)
                inv = stile("inv")
                inv2 = stile("inv2")
                inv3 = stile("inv3")
                thw = stile("thw", pool=ps)
                s32 = stile("s32")
                sh16 = stile("sh16", F16)
                c116 = stile("c116", F16)
                thms = stile("r32", pool=ps)  # alias r32 (dead once th exists)
                k1 = stile("k1", F16)
                k2 = stile("k2", F16)
                sg16 = stile("sg16", F16)
                dwh = stile("dwh", F16)
                s16 = stile("s16", F16)
                inv16 = stile("inv16", F16)

                def bc3(s_ap):
                    # [P,F] -> [P,3,F] plane-broadcast (step-0 middle dim)
                    return s_ap.unsqueeze(1).to_broadcast((P, 3, f))

                wpl = wx_t[:, 0 : 3 * f]  # w planar [3F]
                gpl = gx_t[:, 0 : 3 * f]

                # ---- deinterleave / u ----
                S.activation(v3(wx_t), netp[:, 0:3, :], AFT.Copy)  # w planar (ACT)
                V.tensor_copy(v3(v16), netp[:, 3:6, :])  # v planar (DVE)
                G.tensor_tensor(v3(u16), xp, netp[:, 6:9, :], add)  # u planar (GPSIMD)

                # ---- n2 / scalar chain ----
                S.activation(v3(sq16), v3(wx_t), AFT.Square)
                V.tensor_tensor(n2_16[:], sq16[:, 0:f], sq16[:, f : 2 * f], add)
                V.tensor_tensor(n2_16[:], n2_16[:], sq16[:, 2 * f : 3 * f], add)
                S.activation(r32[:], n2_16[:], AFT.Sqrt)
                S.activation(th[:], r32[:], AFT.Copy, bias=EPS)  # theta = r + eps
                V.reciprocal_approx_fast(out=inv[:], in_=th[:])
                V.add_range_wrap(thw[:], th[:], 0.0, math.pi, 2 * math.pi)
                S.activation(s32[:], thw[:], AFT.Sin)
                # 1-cos(theta) = 2*sin(theta/2)^2, valid through the wrap
                S.activation(sh16[:], thw[:], AFT.Sin, scale=0.5)
                S.activation(c116[:], sh16[:], AFT.Square, scale=math.sqrt(2.0))
                S.activation(s16[:], s32[:], AFT.Copy)  # f16 casts on ACT
                S.activation(inv16[:], inv[:], AFT.Copy)
                S.activation(inv2[:], inv[:], AFT.Square)
                V.tensor_tensor(thms[:], th[:], s32[:], sub)  # theta - sin(theta)
                V.tensor_tensor(inv3[:], inv2[:], inv[:], mul)
                V.tensor_tensor(k1[:], s16[:], inv16[:], mul)  # f16 2x
                V.tensor_tensor(k2[:], c116[:], inv2[:], mul)  # f16 out
                V.tensor_tensor(sg16[:], thms[:], inv3[:], mul)  # f16 out

                # ---- g = k1*u + k2*v ; h = k2*u + sg*v (planar fp16) ----
                V.tensor_tensor(v3(gx_t), v3(u16), bc3(k1[:]), mul)
                V.tensor_tensor(v3(m1_t), v3(v16), bc3(k2[:]), mul)
                V.tensor_tensor(v3(gx_t), v3(gx_t), v3(m1_t), add)
                V.tensor_tensor(v3(h16), v3(u16), bc3(k2[:]), mul)
                V.tensor_tensor(v3(m2_t), v3(v16), bc3(sg16[:]), mul)
                V.tensor_tensor(v3(h16), v3(h16), v3(m2_t), add)

                # ---- extend w,g; cross = w x g (3 full-width ops) ----
                V.tensor_copy(wx_t[:, 3 * f : 5 * f], wx_t[:, 0 : 2 * f])
                V.tensor_copy(gx_t[:, 3 * f : 5 * f], gx_t[:, 0 : 2 * f])
                V.tensor_tensor(
                    cr16[:], wx_t[:, f : 4 * f], gx_t[:, 2 * f : 5 * f], mul
                )
                V.tensor_tensor(
                    pr16[:], wx_t[:, 2 * f : 5 * f], gx_t[:, f : 4 * f], mul
                )
                V.tensor_tensor(cr16[:], cr16[:], pr16[:], sub)

                # ---- dwh = w.h ; m1 = w*dwh ; m2 = n2*h ----
                V.tensor_tensor(pr16[:], wpl, h16[:], mul)
                V.tensor_tensor(dwh[:], pr16[:, 0:f], pr16[:, f : 2 * f], add)
                V.tensor_tensor(dwh[:], dwh[:], pr16[:, 2 * f : 3 * f], add)
                V.tensor_tensor(v3(m1_t), v3(wx_t), bc3(dwh[:]), mul)
                V.tensor_tensor(v3(m2_t), v3(h16), bc3(n2_16[:]), mul)

                # ---- assembly ----
                G.tensor_tensor(v3(a1_t), v3(v16), netp[:, 9:12, :], add)  # v + t
                V.tensor_tensor(v3(m1_t), v3(m1_t), v3(m2_t), sub)  # m1-m2 (a2)
                V.tensor_tensor(v3(a1_t), v3(a1_t), v3(m1_t), add)  # a3
                # final: interleave planar (a3 + cross) into o16 dense layout
                o_pl = o16[:].rearrange("p (f c) -> p c f", c=3)
                G.tensor_tensor(o_pl, v3(a1_t), v3(cr16), add)

                G.dma_start(out=out_r[i], in_=o16[:])  # f16 -> f32 cast store

    nc.compile()
    return nc


_NC_CACHE: dict = {}


def _get_nc():
    if "nc" not in _NC_CACHE:
        _NC_CACHE["nc"] = build_nc()
    return _NC_CACHE["nc"]


def kernel(undeformed_positions: np.ndarray, network_output: np.ndarray) -> np.ndarray:
    pos = np.ascontiguousarray(np.asarray(undeformed_positions, dtype=np.float32))
    net = np.ascontiguousarray(np.asarray(network_output, dtype=np.float32))
    assert pos.shape == (N_TOTAL, 3) and net.shape == (N_TOTAL, 12)

    nc = _get_nc()
    in_maps = [
        {
            "pos": pos[i * NPC : (i + 1) * NPC],
            "net": net[i * NPC : (i + 1) * NPC],
        }
        for i in range(NCORES)
    ]
    res = run_bass_kernel_spmd(nc, in_maps, list(range(NCORES)))
    return np.concatenate([res.results[i]["out"] for i in range(NCORES)], axis=0)


# revision 29
# speedup vs baseline: 1.0206x; 1.0206x over previous
"""Trainium2 Bass kernel for the SE(3) deformation model.

reference math (per point):
    w, v, pivot, t = split(network_output, 4)
    theta = |w| + eps ; wn = w/theta ; vn = v/theta
    R = I + sin(theta) K + (1-cos(theta)) K^2          (K = skew(wn))
    p = (theta I + (1-cos) K + (theta-sin) K^2) vn
    out = R (x + pivot) + p - pivot + t - x

Exact rewrite used here (K~ = skew(w) unnormalized, n2 = |w|^2):
    u  = x + pivot
    k1 = sin(theta)/theta ; k2 = (1-cos(theta))/theta^2
    sg = (theta - sin(theta))/theta^3
    out = K~ (k1 u + k2 v) + K~^2 (k2 u + sg v) + v + t
    with K~^2 a = w (w . a) - n2 a
(algebraically identical to the reference; skew(a)^2 = a a^T - |a|^2 I).

Implementation notes (driven by measured HW rates):
  - Data is processed in chunks of 128*F points; fp16 "planar" layout
    [w0|w1|w2] (three [128,F] planes in one [128,3F] tile) makes every
    bulk op a contiguous step-1 fp16 op -> DVE 2x mode.
  - Cross product via extended tiles [p0|p1|p2|p0|p1]: rotated component
    views are then contiguous [3F] slices, so w x g is 3 full-width ops.
  - Per-point coefficients [128,F] are broadcast over planes with
    step-0 middle-dim APs (measured: free on DVE).
  - Inputs are cast f32->f16 during the DMA load (SWDGE), output cast
    f16->f32 during the store. HBM traffic is unchanged (f32 on the
    DRAM side); SBUF tiles halve.
  - The scalar chain stays f32: theta-sin(theta) cancels catastrophically
    in f16, and 1/theta^2, 1/theta^3 overflow f16 range for the smallest
    theta in a 4M-point gaussian sample.
  - sin inputs are range-reduced with the ADD_RANGE_WRAP custom DVE op
    (ACT sin domain is [-pi,pi]); cos(x) = -sin(x - pi/2) keeps the
    wrapped argument in range for theta < 3.5*pi.
  - Engine split keeps ACT on 1-input work (activations, planar
    deinterleave copy), GPSIMD on three big ops, DVE on the rest.
"""

import math

import numpy as np

import concourse.bacc as bacc
import concourse.mybir as mybir
import concourse.tile as tile
from concourse.alu_op_type import AluOpType
from concourse.bass_utils import run_bass_kernel_spmd

AFT = mybir.ActivationFunctionType
F32 = mybir.dt.float32
F16 = mybir.dt.float16

N_TOTAL = 4194304
NCORES = 8
NPC = N_TOTAL // NCORES  # 524288 points per core
P = 128
F_DEF = 512  # points per partition per chunk
EPS = 1e-6
HALF_PI = float(np.pi / 2)


def build_nc(npc: int = NPC, f: int = F_DEF):
    nchunks = npc // (P * f)
    assert nchunks * P * f == npc

    nc = bacc.Bacc("TRN2", target_bir_lowering=False, debug=False)

    pos = nc.dram_tensor("pos", [npc, 3], F32, kind="ExternalInput")
    net = nc.dram_tensor("net", [npc, 12], F32, kind="ExternalInput")
    out = nc.dram_tensor("out", [npc, 3], F32, kind="ExternalOutput")

    pos_r = pos.ap().rearrange("(n p f) c -> n p (f c)", p=P, f=f)
    net_r = net.ap().rearrange("(n p f) c -> n p (f c)", p=P, f=f)
    out_r = out.ap().rearrange("(n p f) c -> n p (f c)", p=P, f=f)

    V = nc.vector
    G = nc.gpsimd
    S = nc.scalar
    mul, add, sub = AluOpType.mult, AluOpType.add, AluOpType.subtract

    with tile.TileContext(nc) as tc:
        with (
            tc.tile_pool(name="io", bufs=2) as io,
            tc.tile_pool(name="vec", bufs=2) as vec,
            tc.tile_pool(name="sc", bufs=2) as sc,
            tc.tile_pool(name="ps", bufs=2, space="PSUM") as ps,
        ):
            for i in range(nchunks):
                x32 = io.tile([P, 3 * f], F32, tag="x", name="x32")
                net32 = io.tile([P, 12 * f], F32, tag="net", name="net32")
                o16 = io.tile([P, 3 * f], F16, tag="o", name="o16")

                # HWDGE loads (keeps the GPSIMD queue free of DGE work);
                # the deinterleave ops below do the f32->f16 cast for free.
                nc.sync.dma_start(out=x32[:], in_=pos_r[i])
                nc.sync.dma_start(out=net32[:], in_=net_r[i])

                # planar-order views of the interleaved inputs:
                # [P, comp, point] with per-point stride 12 (net) / 3 (pos)
                netp = net32[:].rearrange("p (f c) -> p c f", c=12)
                xp = x32[:].rearrange("p (f c) -> p c f", c=3)

                def v3(t):
                    return t[:, 0 : 3 * f].rearrange("p (c f) -> p c f", c=3)

                # planar tiles ([3F] = concatenated planes); w,g extended to 5F
                wx_t = vec.tile([P, 5 * f], F16, tag="wx", name="wx_t", bufs=3)
                gx_t = vec.tile([P, 5 * f], F16, tag="gx", name="gx_t")
                v16 = vec.tile([P, 3 * f], F16, tag="v16", name="v16", bufs=3)
                u16 = vec.tile([P, 3 * f], F16, tag="u16", name="u16", bufs=3)
                h16 = vec.tile([P, 3 * f], F16, tag="h16", name="h16")
                sq16 = vec.tile([P, 3 * f], F16, tag="sq16", name="sq16", bufs=3)
                pr16 = vec.tile([P, 3 * f], F16, tag="pr16", name="pr16")
                cr16 = vec.tile([P, 3 * f], F16, tag="cr16", name="cr16")
                a1_t = vec.tile([P, 3 * f], F16, tag="a1", name="a1_t")
                m1_t = vec.tile([P, 3 * f], F16, tag="m1", name="m1_t")
                m2_t = vec.tile([P, 3 * f], F16, tag="sq16", name="m2_t", bufs=3)

                def stile(tag, dt=F32, pool=None):
                    return (pool or sc).tile([P, f], dt, tag=tag, name=tag + "_t")

                n2_16 = stile("n2h", F16)
                r32 = stile("r32", pool=ps)
                th = stile("th", pool=ps)
                inv = stile("inv")
                inv2 = stile("inv2")
                inv3 = stile("inv3")
                thw = stile("thw", pool=ps)
                s32 = stile("s32")
                sh16 = stile("sh16", F16)
                c116 = stile("c116", F16)
                thms = stile("r32", pool=ps)  # alias r32 (dead once th exists)
                k1 = stile("k1", F16)
                k2 = stile("k2", F16)
                sg16 = stile("sg16", F16)
                dwh = stile("dwh", F16)
                s16 = stile("s16", F16)
                inv16 = stile("inv16", F16)

                def bc3(s_ap):
                    # [P,F] -> [P,3,F] plane-broadcast (step-0 middle dim)
                    return s_ap.unsqueeze(1).to_broadcast((P, 3, f))

                wpl = wx_t[:, 0 : 3 * f]  # w planar [3F]
                gpl = gx_t[:, 0 : 3 * f]

                # ---- deinterleave / u ----
                S.activation(v3(wx_t), netp[:, 0:3, :], AFT.Copy)  # w planar (ACT)
                V.tensor_copy(v3(v16), netp[:, 3:6, :])  # v planar (DVE)
                G.tensor_tensor(v3(u16), xp, netp[:, 6:9, :], add)  # u planar (GPSIMD)

                # ---- n2 / scalar chain ----
                S.activation(v3(sq16), v3(wx_t), AFT.Square)
                V.tensor_tensor(n2_16[:], sq16[:, 0:f], sq16[:, f : 2 * f], add)
                V.tensor_tensor(n2_16[:], n2_16[:], sq16[:, 2 * f : 3 * f], add)
                S.activation(r32[:], n2_16[:], AFT.Sqrt)
                S.activation(th[:], r32[:], AFT.Copy, bias=EPS)  # theta = r + eps
                V.reciprocal_approx_fast(out=inv[:], in_=th[:])
                V.add_range_wrap(thw[:], th[:], 0.0, math.pi, 2 * math.pi)
                S.activation(s32[:], thw[:], AFT.Sin)
                # 1-cos(theta) = 2*sin(theta/2)^2, valid through the wrap
                S.activation(sh16[:], thw[:], AFT.Sin, scale=0.5)
                S.activation(c116[:], sh16[:], AFT.Square, scale=math.sqrt(2.0))
                S.activation(s16[:], s32[:], AFT.Copy)  # f16 casts on ACT
                S.activation(inv16[:], inv[:], AFT.Copy)
                S.activation(inv2[:], inv[:], AFT.Square)
                V.tensor_tensor(thms[:], th[:], s32[:], sub)  # theta - sin(theta)
                V.tensor_tensor(inv3[:], inv2[:], inv[:], mul)
                V.tensor_tensor(k1[:], s16[:], inv16[:], mul)  # f16 2x
                V.tensor_tensor(k2[:], c116[:], inv2[:], mul)  # f16 out
                V.tensor_tensor(sg16[:], thms[:], inv3[:], mul)  # f16 out

                # ---- g = k1*u + k2*v ; h = k2*u + sg*v (planar fp16) ----
                V.tensor_tensor(v3(gx_t), v3(u16), bc3(k1[:]), mul)
                V.tensor_tensor(v3(m1_t), v3(v16), bc3(k2[:]), mul)
                V.tensor_tensor(v3(gx_t), v3(gx_t), v3(m1_t), add)
                V.tensor_tensor(v3(h16), v3(u16), bc3(k2[:]), mul)
                V.tensor_tensor(v3(m2_t), v3(v16), bc3(sg16[:]), mul)
                V.tensor_tensor(v3(h16), v3(h16), v3(m2_t), add)

                # ---- extend w,g; cross = w x g (3 full-width ops) ----
                V.tensor_copy(wx_t[:, 3 * f : 5 * f], wx_t[:, 0 : 2 * f])
                V.tensor_copy(gx_t[:, 3 * f : 5 * f], gx_t[:, 0 : 2 * f])
                V.tensor_tensor(
                    cr16[:], wx_t[:, f : 4 * f], gx_t[:, 2 * f : 5 * f], mul
                )
                V.tensor_tensor(
                    pr16[:], wx_t[:, 2 * f : 5 * f], gx_t[:, f : 4 * f], mul
                )
                V.tensor_tensor(cr16[:], cr16[:], pr16[:], sub)

                # ---- dwh = w.h ; m1 = w*dwh ; m2 = n2*h ----
                V.tensor_tensor(pr16[:], wpl, h16[:], mul)
                V.tensor_tensor(dwh[:], pr16[:, 0:f], pr16[:, f : 2 * f], add)
                V.tensor_tensor(dwh[:], dwh[:], pr16[:, 2 * f : 3 * f], add)
                V.tensor_tensor(v3(m1_t), v3(wx_t), bc3(dwh[:]), mul)
                V.tensor_tensor(v3(m2_t), v3(h16), bc3(n2_16[:]), mul)

                # ---- assembly ----
                G.tensor_tensor(v3(a1_t), v3(v16), netp[:, 9:12, :], add)  # v + t
                V.tensor_tensor(v3(m1_t), v3(m1_t), v3(m2_t), sub)  # m1-m2 (a2)
                V.tensor_tensor(v3(a1_t), v3(a1_t), v3(m1_t), add)  # a3
                # final: interleave planar (a3 + cross) into o16 dense layout
                o_pl = o16[:].rearrange("p (f c) -> p c f", c=3)
                G.tensor_tensor(o_pl, v3(a1_t), v3(cr16), add)

                G.dma_start(out=out_r[i], in_=o16[:])  # f16 -> f32 cast store

    nc.compile()
    return nc


_NC_CACHE: dict = {}


def _get_nc():
    if "nc" not in _NC_CACHE:
        _NC_CACHE["nc"] = build_nc()
    return _NC_CACHE["nc"]


def kernel(undeformed_positions: np.ndarray, network_output: np.ndarray) -> np.ndarray:
    pos = np.ascontiguousarray(np.asarray(undeformed_positions, dtype=np.float32))
    net = np.ascontiguousarray(np.asarray(network_output, dtype=np.float32))
    assert pos.shape == (N_TOTAL, 3) and net.shape == (N_TOTAL, 12)

    nc = _get_nc()
    in_maps = [
        {
            "pos": pos[i * NPC : (i + 1) * NPC],
            "net": net[i * NPC : (i + 1) * NPC],
        }
        for i in range(NCORES)
    ]
    res = run_bass_kernel_spmd(nc, in_maps, list(range(NCORES)))
    return np.concatenate([res.results[i]["out"] for i in range(NCORES)], axis=0)
